# revision 168
# baseline (speedup 1.0000x reference)
"""Trainium2 Bass kernel for nn_MoDBlock (mixture-of-depths transformer block).

Sharding: data-parallel over batch B=8 across the 8 NeuronCores (one batch row
per core; routing/gather/scatter are per-row independent). Everything runs
on-device per core:

  logits  = x @ Wr                      (DVE fused mul+reduce during x load)
  thr     = 512th-largest logit         (gpsimd kth_largest, exact: desc[511])
  sel     = ascending-index compaction  (gpsimd sparse_gather on masked iota)
  tok     = dma_gather(x, sel)          (per-chunk so LN1 starts early)
  w       = recomputed as tok @ Wr      (cheaper than a second sparse_gather)
  block   = pre-LN attention + SwiGLU MLP. All projections (Wqkv, Wo, W1,
            W3, W2) run as fp8e4 DoubleRow matmuls (2x PE throughput, half
            the weight HBM traffic); activations are pre-scaled by AS=8 and
            weights by WS=64 before fp8 quantization, descaled 1/512 on
            PSUM eviction. Attention scores/probs stay bf16; softmax without
            max-subtraction (|scores/8| < 3 at this operator's scale); only
            the diagonal 128x128 block is causal-masked (off-diagonal live
            blocks are all-ones). v is produced token-major straight from
            the QKV matmul (stationary-swapped) with a fused ones column, so
            the attention inner loop has no transposes and the softmax
            denominator accumulates in the same matmul as o.
  out     = 8 tiles copied through during the routing window; 24 tiles
            retained in SBUF as bf16 and written back during block compute;
            then per-chunk dma_scatter_add(out, proc * w, sel)

Engine balance: LN/exp/silu and PSUM evictions on Act, elementwise and
reciprocals on DVE, copy-through converts on gpsimd, everything batched
into paired/quad ops to minimize instruction count (HW is overhead-bound).
Weights load as single large partition-major packed DMAs.

Host-side preprocessing: fp8 pre-cast + DoubleRow pair-major packing, LN
gains folded into Wqkv/W1/W3 rows.
"""

import os
from contextlib import ExitStack
import numpy as np
import ml_dtypes

import concourse.bass as bass
import concourse.mybir as mybir
import concourse.tile as tile
from concourse import bacc, masks
from concourse.bass_utils import run_bass_kernel_spmd

F32 = mybir.dt.float32
BF16 = mybir.dt.bfloat16
FP8 = mybir.dt.float8e4
I16 = mybir.dt.int16
I32 = mybir.dt.int32
U32 = mybir.dt.uint32
AF = mybir.ActivationFunctionType
ALU = mybir.AluOpType
DR = mybir.MatmulPerfMode.DoubleRow

AS = 8.0                     # activation fp8 pre-scale
WS = 64.0                    # weight fp8 pre-scale
SC = 1.0 / (AS * WS)         # descale factor applied on PSUM eviction

B, T, C = 8, 4096, 1024
H, DH, FF = 16, 64, 4096
K = 512                      # routed tokens per batch row
EPS = 1e-5
NT = T // 128                # 32 x-tiles
NI = K // 128                # 4 reduced-seq token chunks
NCC = C // 128               # 8 feature chunks
NFC = FF // 128              # 32 ffn chunks
N_CORES = 8


KSTOP = int(os.environ.get("KSTOP", "99"))
# repeat the whole kernel body inside the program (timing: difference the
# wall time of REPEAT=n vs REPEAT=1 to cancel dispatch overhead)
REPEAT = int(os.environ.get("BASS_REPEAT", "1"))


def build(nc, tc, es_outer):
    # fp8 weights in DoubleRow pair-major layouts, partition-major packed so
    # each full weight loads with a single large DMA
    tensors = dict(
        x_d=nc.dram_tensor("x", (T, C), F32, kind="ExternalInput").ap(),
        wr_d=nc.dram_tensor("wr", (1, C), F32, kind="ExternalInput").ap(),
        wqkv_d=nc.dram_tensor("wqkv", (128, 4, 2, 3 * C), FP8,
                              kind="ExternalInput").ap(),
        wo_d=nc.dram_tensor("wo", (64, 8, 2, C), FP8,
                            kind="ExternalInput").ap(),
        w1_d=nc.dram_tensor("w1", (4, 128, 4, 2, 1024), FP8,
                            kind="ExternalInput").ap(),
        w3_d=nc.dram_tensor("w3", (4, 128, 4, 2, 1024), FP8,
                            kind="ExternalInput").ap(),
        w2_d=nc.dram_tensor("w2", (128, 16, 2, C), FP8,
                            kind="ExternalInput").ap(),
        out_d=nc.dram_tensor("out", (T, C), F32, kind="ExternalOutput").ap(),
    )
    for _rep in range(REPEAT):
        with ExitStack() as es:
            _body(nc, tc, es, **tensors)


def _body(nc, tc, es, x_d, wr_d, wqkv_d, wo_d, w1_d, w3_d, w2_d, out_d):

    const = es.enter_context(tc.tile_pool(name="const", bufs=1))
    ident = const.tile([128, 128], BF16)
    masks.make_identity(nc, ident[:])
    ones65 = const.tile([65, 128], BF16)
    nc.vector.memset(ones65[:], AS)    # folds the o16 fp8 pre-scale into pz
    # diagonal causal mask: cmask[p, i] = 1.0 if i >= p else 0 (the only
    # masked block per j-chunk is the diagonal one; off-diagonal live blocks
    # are all-ones)
    cmask2 = const.tile([128, 2, 128], BF16, name="cmask2")
    nc.gpsimd.memset(cmask2[:], 1.0)
    for j in range(2):
        nc.gpsimd.affine_select(
            out=cmask2[:, j, :], in_=cmask2[:, j, :], compare_op=ALU.is_ge,
            fill=0.0, base=0, channel_multiplier=-1, pattern=[[1, 128]],
        )
    # register const APs used as activation biases (Exp/Silu need 0.0, Sqrt
    # uses EPS); bass converts float biases via nc.const_aps
    for val in (0.0, EPS / (AS * AS)):
        cz = const.tile([128, 1], F32, name=f"constap_{val}")
        nc.vector.memset(cz[:], val)
        nc.const_aps.aps[(F32, val)] = cz[:]
    wr_b = const.tile([128, C], F32)
    nc.sync.dma_start(out=wr_b[0:1, :], in_=wr_d[:, :])
    nc.gpsimd.partition_broadcast(wr_b[:], wr_b[0:1, :])
    logit_sb = const.tile([128, NT], F32)       # token t = col*128 + p

    # persistent activations (lnp is created first so LN scratch never
    # waits on another pool's release)
    lnp = es.enter_context(tc.tile_pool(name="lnp", bufs=3))
    hb = es.enter_context(tc.tile_pool(name="hb", bufs=2))
    py = es.enter_context(tc.tile_pool(name="py", bufs=1))
    y0 = py.tile([128, NI, C], F32)             # gathered rows, token-major
    y1 = py.tile([128, NI, C], BF16)            # after attention residual
    swr = y0                                    # scatter source (reuses y0:
    #                                             y0's last read is stage 6)
    idx128 = py.tile([128, 32], I16)
    w128 = py.tile([128, NI], F32)

    # persistent weight/output pools (created before pq_stack for LIFO)
    w13p = es.enter_context(tc.tile_pool(name="w13p", bufs=2))
    obuf = es.enter_context(tc.tile_pool(name="obuf", bufs=1))
    # attention outputs packed head-major in one tile so Wo runs DoubleRow
    obig = obuf.tile([64, H, K], FP8, name="obig")

    # pq_stack pools release right after attention; xkeep lives there since
    # its last read is the final copy-through write
    pq_stack = ExitStack()
    pqx = pq_stack.enter_context(tc.tile_pool(name="pqx", bufs=1))
    NKEEP = NT - 8                              # tiles retained in SBUF
    xkeep = pqx.tile([128, NKEEP, C], BF16, name="xkeep")

    # ---------------- stage 1: x load, logits, bf16 x retention ------------
    # the first NKEEP tiles are retained in SBUF as bf16 and written to out
    # during block compute; the last 8 are copied through directly — their
    # writes are emitted after the routing DMAs so those aren't delayed on
    # the in-order SP queue
    x_stack = ExitStack()
    xio = x_stack.enter_context(tc.tile_pool(name="xio", bufs=6))
    junkp = x_stack.enter_context(tc.tile_pool(name="junkp", bufs=2))
    held = []
    for tp in range(NT // 2):
        # two token-tiles per DMA (3-dim AP) to halve issue overhead
        xt = xio.tile([128, 2, C], F32, tag="xt")
        nc.sync.dma_start(
            out=xt[:],
            in_=x_d.rearrange("(n p) c -> p n c", p=128)[:,
                              2 * tp:2 * tp + 2, :])
        for j in range(2):
            t = 2 * tp + j
            junk = junkp.tile([128, C], BF16, tag="junk")
            nc.vector.scalar_tensor_tensor(
                out=junk[:], in0=xt[:, j, :], scalar=1.0, in1=wr_b[:],
                op0=ALU.mult, op1=ALU.mult,
                accum_out=logit_sb[:, t:t + 1])
            if t >= NKEEP:
                held.append((t, xt, j))
            else:
                nc.scalar.activation(xkeep[:, t, :], xt[:, j, :], AF.Copy)

    if KSTOP == 1:
        nc.sync.dma_start(out=out_d[0:128, 0:NT], in_=logit_sb[:])
        x_stack.close()
        pq_stack.close()
        return

    # ---------------- stage 2: routing ----------------
    rt_stack = ExitStack()
    rt = rt_stack.enter_context(tc.tile_pool(name="route", bufs=1))
    kth = rt.tile([1, 2], F32)
    # quantile s.t. k_adj = floor(0.1246*4095) = 510 -> out[0,1] = desc[511]
    nc.gpsimd.kth_largest(kth[:], logit_sb[:], n_per_lane=NT, k=510,
                          quantile=1.0 - 0.1246)
    thr16 = rt.tile([16, 1], F32)
    nc.gpsimd.partition_broadcast(thr16[:], kth[0:1, 1:2])

    # rearrange logits to 16-wrapped: l16[p, 8f+g] = logit_sb[16g+p, f]
    l16 = rt.tile([16, 256], F32)
    for g in range(8):
        nc.sync.dma_start(out=l16[:, g::8],
                          in_=logit_sb[g * 16:(g + 1) * 16, :])
    m01 = rt.tile([16, 256], F32)
    nc.vector.tensor_scalar(out=m01[:], in0=l16[:], scalar1=thr16[:, 0:1],
                            scalar2=None, op0=ALU.is_ge)
    iota_i = rt.tile([16, 256], I32)
    nc.gpsimd.iota(iota_i[:], pattern=[[16, 256]], base=1, channel_multiplier=1)
    iota_f = rt.tile([16, 256], F32)
    nc.vector.tensor_copy(iota_f[:], iota_i[:])
    selm = rt.tile([16, 256], F32)   # j+1 if selected else 0 ... then -1
    nc.vector.tensor_tensor(out=selm[:], in0=m01[:], in1=iota_f[:], op=ALU.mult)
    nc.vector.tensor_scalar_add(selm[:], selm[:], -1.0)
    idxw = rt.tile([16, 32], F32)
    nfound = rt.tile([1, 1], U32)
    nc.gpsimd.sparse_gather(idxw[:], selm[:], num_found=nfound[:])
    idxw16 = rt.tile([16, 32], I16)
    nc.vector.tensor_copy(idxw16[:], idxw[:])

    # replicate the 16-wrapped indices to all 8 partition groups directly
    # (SBUF -> SBUF; no DRAM bounce)
    for g in range(8):
        nc.sync.dma_start(out=idx128[g * 16:(g + 1) * 16, :], in_=idxw16[:])
    rt_stack.close()
    # copy-through writes for the held tiles fill the SP queue while the
    # gather/LN1 pipeline spins up (one paired DMA per held tile-pair)
    for t, xt, j in held:
        if j == 0:
            nc.sync.dma_start(
                out=out_d.rearrange("(n p) c -> p n c", p=128)[:,
                                    t:t + 2, :],
                in_=xt[:])
    x_stack.close()

    # ---------------- stage 3: gather + LN1 + transpose ----------------
    # per-chunk gathers so LN1 of chunk 0 starts 3 chunks earlier; router
    # weights recomputed from the gathered rows on gpsimd (cheaper than a
    # second sparse_gather, and off DVE's LN1 path)
    for i in range(NI):
        nc.gpsimd.dma_gather(out_ap=y0[:, i:i + 1, :], in_ap=x_d[:, :],
                             idxs_ap=idx128[:, 8 * i:8 * i + 8],
                             num_idxs=128, num_idxs_reg=128, elem_size=C)

    if KSTOP == 3:
        for c in range(NI):
            nc.sync.dma_start(out=out_d[c * 128:(c + 1) * 128, :],
                              in_=y0[:, c, :])
        nc.sync.dma_start(out=out_d[512:640, 0:NI], in_=w128[:])
        return

    def ln_tokmajor(src, dst):
        # LayerNorm over free dim (C) of token-major [128, C] f32 -> bf16,
        # scaled by AS (the fp8 activation pre-scale)
        st = lnp.tile([128, 2, 6], F32, tag="bnst")
        nc.vector.bn_stats(st[:, 0, :], src[:, 0:512])
        nc.vector.bn_stats(st[:, 1, :], src[:, 512:1024])
        ag = lnp.tile([128, 2], F32, tag="bnag")
        nc.vector.bn_aggr(ag[:], st[:])
        # sd = sqrt(var + eps)/AS, so its reciprocal is the fp8-scaled rstd
        sd = lnp.tile([128, 1], F32, tag="sd")
        nc.scalar.activation(sd[:], ag[:, 1:2], AF.Sqrt,
                             bias=EPS / (AS * AS), scale=1.0 / (AS * AS))
        rs8 = lnp.tile([128, 1], F32, tag="rs8")
        nc.vector.reciprocal(rs8[:], sd[:])
        nb = lnp.tile([128, 1], F32, tag="nb")
        nc.vector.scalar_tensor_tensor(out=nb[:], in0=ag[:, 0:1], scalar=-1.0,
                                       in1=rs8[:], op0=ALU.mult, op1=ALU.mult)
        nc.scalar.activation(dst[:], src[:], AF.Identity, bias=nb[:],
                             scale=rs8[:])

    # QKV activation pool: created at stage 3 (after stage-1 scratch is
    # released), freed with pq_stack right after attention
    pqkv = pq_stack.enter_context(tc.tile_pool(name="pqkv", bufs=1))
    # hT pairs: feature-major activations for DoubleRow fp8 QKV
    hTall = pqkv.tile([128, 4, 2, K], FP8, name="hTall")
    hT = [hTall[:, pc] for pc in range(4)]
    # q/k feature-major: [:, c, 0] = q head-pair c, [:, c, 1] = k pair c
    qkall = pqkv.tile([128, 8, 2, K], BF16, name="qkall")
    # v token-major, head-major free layout with a fused ones column per
    # head (row 64 of each head's po_t accumulates Z in the same matmul)
    vT = pqkv.tile([128, NI, H, 65], BF16, name="vT")
    nc.vector.memset(vT[:, :, :, 64:65], 1.0)

    # ------- stage 3+4: LN1 + transpose + QKV (fp8 DoubleRow), fused -------
    # v only needs hT's per-chunk slice, so its matmuls interleave into the
    # LN1 loop and fill PE while the LN pipeline spins up. pv and pq psum
    # tiles share one pool (identical 2-bank shape).
    with tc.tile_pool(name="wqkvp", bufs=1) as wqkvp, \
         tc.tile_pool(name="tpsum", bufs=4, space="PSUM") as tpsum, \
         tc.tile_pool(name="qvp", bufs=2, space="PSUM") as qvp:
        wqall = wqkvp.tile([128, 4, 2, 3 * C], FP8, tag="wqall")
        nc.sync.dma_start(out=wqall[:], in_=wqkv_d[:])
        wq = [wqall[:, pc] for pc in range(4)]
        # Wo loads right behind wqkv so the DMA lands before stage 6
        woall = obuf.tile([64, 8, 2, C], FP8, name="woall")
        nc.sync.dma_start(out=woall[:], in_=wo_d[:])
        wo_sb = [woall[:, cd] for cd in range(8)]

        for i in range(NI):
            hti = hb.tile([128, C], BF16, tag="hm")
            ln_tokmajor(y0[:, i, :], hti[:])
            for g in range(2):
                # 4 transposes per PSUM bank, one wide copy out
                pt = tpsum.tile([128, 4, 128], BF16, tag="tp")
                for k_ in range(4):
                    cc = 4 * g + k_
                    nc.tensor.transpose(pt[:, k_, :],
                                        hti[:, cc * 128:(cc + 1) * 128],
                                        ident[:])
                nc.vector.tensor_copy(
                    hTall[:, 2 * g:2 * g + 2, :, i * 128:(i + 1) * 128],
                    pt[:])
            # v token-major (stationary-swapped) for this chunk
            pv = qvp.tile([128, 2, K], F32, tag="pqv")
            for vh in range(2):
                for pc in range(4):
                    nc.tensor.matmul(
                        pv[:, vh, :],
                        hT[pc][:, :, i * 128:(i + 1) * 128],
                        wq[pc][:, :, 2 * C + vh * 512:2 * C + (vh + 1) * 512],
                        start=(pc == 0), stop=(pc == 3), perf_mode=DR)
            nc.scalar.activation(vT[:, i, :, 0:64], pv[:], AF.Copy, scale=SC)

        # router weights recomputed from the gathered rows (cheaper than a
        # second sparse_gather); after LN1 so bn_stats isn't queue-delayed
        for i in range(NI):
            junk2 = lnp.tile([128, C], BF16, tag="junk2")
            nc.vector.scalar_tensor_tensor(
                out=junk2[:], in0=y0[:, i, :], scalar=1.0, in1=wr_b[:],
                op0=ALU.mult, op1=ALU.mult,
                accum_out=w128[:, i:i + 1])

        # q,k feature-major, one (q_c, k_c) head-pair per 2-bank PSUM with a
        # single paired eviction; pair c ready after one eviction
        for c in range(8):
            pq = qvp.tile([128, 2, K], F32, tag="pqv")
            for half, m in ((0, c), (1, 8 + c)):
                for pc in range(4):
                    nc.tensor.matmul(pq[:, half, :],
                                     wq[pc][:, :, m * 128:(m + 1) * 128],
                                     hT[pc][:], start=(pc == 0),
                                     stop=(pc == 3), perf_mode=DR)
            nc.vector.tensor_scalar_mul(qkall[:, c], pq[:], SC)

    if KSTOP == 4:
        for m in range(16):
            nc.gpsimd.dma_start(out=out_d[m * 128:(m + 1) * 128, 0:K],
                                in_=qkv_sb[m][:])
        return

    # deferred x copy-through from the SBUF bf16 retention: gpsimd converts
    # back to f32 (idle engine), DMA writes out. Interleaved into the
    # attention loop so the writes overlap block compute.
    xcp = pq_stack.enter_context(tc.tile_pool(name="xcp", bufs=2))

    def emit_xout(t2):
        # one convert + one DMA per PAIR of retained tiles
        xt2 = xcp.tile([128, 2, C], F32, tag="xc")
        nc.gpsimd.tensor_copy(xt2[:], xkeep[:, 2 * t2:2 * t2 + 2, :])
        nc.sync.dma_start(
            out=out_d.rearrange("(n p) c -> p n c", p=128)[:,
                                2 * t2:2 * t2 + 2, :],
            in_=xt2[:])

    # ---------------- stage 5: attention ----------------
    # layouts: q = qkv chunks 0-7, k = 8-15, v = 16-23; head h lives in chunk
    # h//2 at partition offset 64*(h%2).
    w13_pre = None

    def load_w13_group(fg):
        t1 = w13p.tile([128, 4, 2, 1024], FP8, tag="w1g")
        nc.sync.dma_start(out=t1[:], in_=w1_d[fg])
        t3 = w13p.tile([128, 4, 2, 1024], FP8, tag="w3g")
        nc.sync.dma_start(out=t3[:], in_=w3_d[fg])
        return ([t1[:, pc] for pc in range(4)],
                [t3[:, pc] for pc in range(4)])

    with tc.tile_pool(name="apool", bufs=3) as ap_, \
         tc.tile_pool(name="eapool", bufs=5) as eap_, \
         tc.tile_pool(name="azpool", bufs=2) as az_, \
         tc.tile_pool(name="spsum", bufs=2, space="PSUM") as spsum, \
         tc.tile_pool(name="opsum", bufs=3, space="PSUM") as opsum, \
         tc.tile_pool(name="zpsum", bufs=1, space="PSUM") as zpsum:
        def emit_scores(c, jcs, att):
            # scores + exp + diagonal causal mask for head pair (2c, 2c+1):
            # both heads share a 2-bank PSUM so exp and mask run as single
            # double-width ops
            for jc in jcs:
                # causal skip: queries i < jc*128 are fully masked for this
                # j-chunk; compute only the live i-range. Only the diagonal
                # 128-col block needs the causal mask multiply.
                lo = jc * 128
                ps = spsum.tile([128, 2, K], F32, tag="ps")
                for hh in range(2):
                    po = 64 * hh
                    nc.tensor.matmul(
                        ps[:, hh, lo:],
                        qkall[po:po + 64, c, 1, jc * 128:(jc + 1) * 128],
                        qkall[po:po + 64, c, 0, lo:],
                        start=True, stop=True)
                ea = eap_.tile([128, 2, K], BF16, tag="ea")
                nc.scalar.activation(ea[:, :, lo:], ps[:, :, lo:], AF.Exp,
                                     scale=0.125)
                nc.vector.tensor_tensor(out=ea[:, :, lo:lo + 128],
                                        in0=ea[:, :, lo:lo + 128],
                                        in1=cmask2[:], op=ALU.mult)
                att.append(ea)
            return att

        def emit_pz(c, orw2, zrb2):
            # deferred by one pair-iteration so the Z-reciprocal chain
            # (Act evict + 2 serial DVE reciprocals) never stalls PE
            for hh in range(2):
                pz = zpsum.tile([64, K], F32, tag="pz")
                nc.tensor.matmul(pz[:], ones65[64:65, 0:64],
                                 zrb2[hh][64:65, :], start=True, stop=True)
                nc.vector.tensor_tensor(out=obig[:, 2 * c + hh, :],
                                        in0=orw2[:, hh, :], in1=pz[:],
                                        op=ALU.mult)

        att_pipe = emit_scores(0, range(NI), [])
        for c in range(H // 2):
            att = att_pipe
            # first two j-chunks of the next pair go ahead of this pair's
            # v-path so ea is ready when the next po_t group starts
            att_pipe = []
            if c + 1 < H // 2:
                emit_scores(c + 1, (0, 1), att_pipe)
            po2 = []
            for hh in range(2):
                h = 2 * c + hh
                po_t = opsum.tile([65, K], F32, tag="po")
                for jc in range(NI):
                    lo = jc * 128
                    nc.tensor.matmul(po_t[:, lo:], vT[:, jc, h, :],
                                     att[jc][:, hh, lo:],
                                     start=(jc == 0), stop=(jc == 3))
                po2.append(po_t)
            orw2 = ap_.tile([64, 2, K], BF16, tag="orw")
            zrb2 = []
            for hh in range(2):
                po_t = po2[hh]
                # evict raw o (divide by Z after broadcast)
                nc.scalar.activation(orw2[:, hh, :], po_t[0:64, :], AF.Copy)
                # Z strip at partition 64 of po_t; reciprocal in-place in
                # PSUM (lane-aligned), then evict to bf16
                nc.vector.reciprocal(po_t[64:65, :], po_t[64:65, :])
                zrb = az_.tile([65, K], BF16, tag="zrb")
                nc.vector.tensor_copy(zrb[64:65, :], po_t[64:65, :])
                zrb2.append(zrb)
            # remaining score matmuls of the next pair fill PE while the
            # Z-reciprocal chain completes
            if c + 1 < H // 2:
                emit_scores(c + 1, (2, 3), att_pipe)
            emit_pz(c, orw2, zrb2)
            # 12 retained tile-pairs over the 8 pair-iterations; W1/W3
            # group 0 prefetches so stage 8 starts immediately
            if c < 4:
                emit_xout(2 * c)
                emit_xout(2 * c + 1)
            else:
                emit_xout(4 + c)
            if c == 4:
                w13_pre = load_w13_group(0)

    if KSTOP == 5:
        for h in range(H):
            nc.gpsimd.dma_start(out=out_d[h * 64:(h + 1) * 64, 0:K],
                                in_=obig[:, h, :])
        return
    pq_stack.close()

    # -------- stage 6+7: Wo + residual, LN2 + transpose (interleaved) ------
    pmlp = es.enter_context(tc.tile_pool(name="pmlp", bufs=1))
    mTall = pmlp.tile([128, 4, 2, K], FP8, name="mTall")
    mT = [mTall[:, pc] for pc in range(4)]
    h2 = [pmlp.tile([128, 2, K], FP8, name=f"h2{pc}") for pc in range(16)]
    with tc.tile_pool(name="aopsum", bufs=2, space="PSUM") as aopsum, \
         tc.tile_pool(name="tpsum2", bufs=4, space="PSUM") as tpsum2:
        def emit_wo(i):
            pao = aopsum.tile([128, C], F32, tag="pao")
            for cd in range(8):           # head pairs, fp8 DoubleRow
                for nh in range(2):
                    nc.tensor.matmul(
                        pao[:, nh * 512:(nh + 1) * 512],
                        obig[:, 2 * cd:2 * cd + 2, i * 128:(i + 1) * 128],
                        wo_sb[cd][:, :, nh * 512:(nh + 1) * 512],
                        start=(cd == 0), stop=(cd == 7), perf_mode=DR)
            nc.vector.scalar_tensor_tensor(out=y1[:, i, :], in0=pao[:],
                                           scalar=SC, in1=y0[:, i, :],
                                           op0=ALU.mult, op1=ALU.add)

        emit_wo(0)
        for i in range(NI):
            if i + 1 < NI:
                emit_wo(i + 1)
            mti = hb.tile([128, C], BF16, tag="hm")
            ln_tokmajor(y1[:, i, :], mti[:])
            for g in range(2):
                pt = tpsum2.tile([128, 4, 128], BF16, tag="tp2")
                for k_ in range(4):
                    cc = 4 * g + k_
                    nc.tensor.transpose(pt[:, k_, :],
                                        mti[:, cc * 128:(cc + 1) * 128],
                                        ident[:])
                nc.scalar.activation(
                    mTall[:, 2 * g:2 * g + 2, :, i * 128:(i + 1) * 128],
                    pt[:], AF.Copy)

    if KSTOP == 6:
        for c in range(NI):
            nc.sync.dma_start(out=out_d[c * 128:(c + 1) * 128, :],
                              in_=y1[:, c, :])
        return

    # ---------------- stage 8: W1/W3 + SwiGLU (fp8 DoubleRow) --------------
    US_SCALE = AS * SC * SC   # us carries AS and both PSUM descales
    w2p = es.enter_context(tc.tile_pool(name="w2p", bufs=1))
    w2_sb = []
    with tc.tile_pool(name="upsum", bufs=2, space="PSUM") as upsum, \
         tc.tile_pool(name="gpsum", bufs=2, space="PSUM") as gpsum, \
         tc.tile_pool(name="sbuf8", bufs=3) as sbuf8:
        for fg in range(4):               # groups of 8 ffn chunks
            w1g, w3g = w13_pre if fg == 0 else load_w13_group(fg)
            if fg == 2:
                # W2 load queued behind the last-but-one W1/W3 group: lands
                # during fg=2/3 compute, ready for stage 9
                w2all = w2p.tile([128, 16, 2, C], FP8, name="w2all")
                nc.sync.dma_start(out=w2all[:], in_=w2_d[:])
                w2_sb.extend(w2all[:, pc] for pc in range(16))
            for fp in range(4):           # ffn chunk PAIRS: one silu + one
                f0 = fg * 8 + 2 * fp      # h2 op per pair (2-bank PSUMs)
                pu = upsum.tile([128, 2, K], F32, tag="pu")
                pg = gpsum.tile([128, 2, K], F32, tag="pg")
                for half in range(2):
                    fi = 2 * fp + half
                    for pc in range(4):
                        nc.tensor.matmul(
                            pu[:, half, :],
                            w1g[pc][:, :, fi * 128:(fi + 1) * 128],
                            mT[pc][:], start=(pc == 0),
                            stop=(pc == 3), perf_mode=DR)
                    for pc in range(4):
                        nc.tensor.matmul(
                            pg[:, half, :],
                            w3g[pc][:, :, fi * 128:(fi + 1) * 128],
                            mT[pc][:], start=(pc == 0),
                            stop=(pc == 3), perf_mode=DR)
                us = sbuf8.tile([128, 2, K], BF16, tag="us")
                nc.scalar.activation(us[:], pu[:], AF.Silu, scale=SC)
                nc.vector.scalar_tensor_tensor(
                    out=h2[f0 // 2][:], in0=pg[:], scalar=AS * SC,
                    in1=us[:], op0=ALU.mult, op1=ALU.mult)

    if KSTOP == 8:
        for f in range(8):
            nc.gpsimd.dma_start(out=out_d[f * 128:(f + 1) * 128, 0:K],
                                in_=h2[f // 2][:, f % 2, :])
        return

    # ---------------- stage 9: W2 (fp8 DoubleRow) + residual + w-scale -----
    with tc.tile_pool(name="mpsum", bufs=2, space="PSUM") as mpsum, \
         tc.tile_pool(name="y2buf", bufs=2) as y2buf:
        for i in range(NI):
            pm = mpsum.tile([128, C], F32, tag="pm")
            for pc in range(16):
                for nh in range(2):
                    nc.tensor.matmul(
                        pm[:, nh * 512:(nh + 1) * 512],
                        h2[pc][:, :, i * 128:(i + 1) * 128],
                        w2_sb[pc][:, :, nh * 512:(nh + 1) * 512],
                        start=(pc == 0), stop=(pc == 15), perf_mode=DR)
            y2t = y2buf.tile([128, C], F32, tag="y2")
            nc.vector.scalar_tensor_tensor(out=y2t[:], in0=pm[:], scalar=SC,
                                           in1=y1[:, i, :], op0=ALU.mult,
                                           op1=ALU.add)
            nc.scalar.activation(swr[:, i, :], y2t[:], AF.Copy,
                                 scale=w128[:, i:i + 1])
            # per-chunk scatter-add: the first 3 overlap stage-9 compute
            nc.gpsimd.dma_scatter_add(out_ap=out_d[:, :],
                                      in_ap=swr[:, i:i + 1, :],
                                      idxs_ap=idx128[:, 8 * i:8 * i + 8],
                                      num_idxs=128, num_idxs_reg=128,
                                      elem_size=C)


_CACHE = {}


def _get_compiled():
    if "nc" in _CACHE:
        return _CACHE["nc"]
    from contextlib import ExitStack
    nc = bacc.Bacc("TRN2", target_bir_lowering=False, debug=False)
    with tile.TileContext(nc) as tc:
        with ExitStack() as es:
            build(nc, tc, es)
    nc.compile()
    _CACHE["nc"] = nc
    return nc


def _fp8_pairs(w, n_groups=1):
    """(R, N) f32 -> fp8 DoubleRow pair-major (R/256, 128, 2, N), with the
    free dim optionally split into n_groups contiguous blocks:
    (R/256 * n_groups, 128, 2, N/n_groups)."""
    R, N = w.shape
    q = np.clip(w * WS_H, -240, 240).astype(ml_dtypes.float8_e4m3)
    q = q.reshape(R // 256, 2, 128, n_groups, N // n_groups)
    q = q.transpose(0, 3, 2, 1, 4)        # [pc, fg, p, pair, n]
    return np.ascontiguousarray(q.reshape(-1, 128, 2, N // n_groups))


WS_H = 64.0   # host-side weight fp8 pre-scale (must match kernel WS)


def make_in_maps(inputs):
    x = np.asarray(inputs["x"], dtype=np.float32)          # (8, 4096, 1024)
    Wr = np.asarray(inputs["Wr"], dtype=np.float32)
    ln1_g = np.asarray(inputs["ln1_g"], dtype=np.float32)
    ln2_g = np.asarray(inputs["ln2_g"], dtype=np.float32)
    bf = ml_dtypes.bfloat16
    wqkv = np.ascontiguousarray(
        _fp8_pairs(np.asarray(inputs["Wqkv"], np.float32)
                   * ln1_g[:, None]).transpose(1, 0, 2, 3))
    wo = np.clip(np.asarray(inputs["Wo"], np.float32) * WS_H,
                 -240, 240).astype(ml_dtypes.float8_e4m3)
    wo = np.ascontiguousarray(
        wo.reshape(8, 2, 64, 1024).transpose(2, 0, 1, 3))
    w1 = np.ascontiguousarray(
        _fp8_pairs(np.asarray(inputs["W1"], np.float32) * ln2_g[:, None],
                   n_groups=4)
        .reshape(4, 4, 128, 2, 1024).transpose(1, 2, 0, 3, 4))
    w3 = np.ascontiguousarray(
        _fp8_pairs(np.asarray(inputs["W3"], np.float32) * ln2_g[:, None],
                   n_groups=4)
        .reshape(4, 4, 128, 2, 1024).transpose(1, 2, 0, 3, 4))
    w2 = np.ascontiguousarray(
        _fp8_pairs(np.asarray(inputs["W2"], np.float32))
        .transpose(1, 0, 2, 3))
    shared = {
        "wr": np.ascontiguousarray(Wr[None, :]),
        "wqkv": wqkv,
        "wo": np.ascontiguousarray(wo),
        "w1": w1,
        "w3": w3,
        "w2": w2,
    }
    return [{"x": np.ascontiguousarray(x[b]), **shared} for b in range(B)]


def kernel(**inputs):
    nc = _get_compiled()
    in_maps = make_in_maps(inputs)
    res = run_bass_kernel_spmd(nc, in_maps, core_ids=list(range(N_CORES)))
    _CACHE["last_results"] = res
    out = np.stack([res.results[b]["out"] for b in range(B)], axis=0)
    return out.astype(np.float32)

